# revision 55
# baseline (speedup 1.0000x reference)
"""Trainium2 Bass kernel: banded additive attention (window 64), v2.3.

reference semantics (B=4, T=1024, D=512, U=32, WIDTH=64):
  q = x @ Wt ; k = x @ Wx
  e[b,t,j] = exp(Wa . tanh(q[b,t]+k[b,j]+bh) + ba) for j in [t-32, t+31]
  a = e / (sum_j e + 1e-7) ; v = a @ x

Sharding: 8 NeuronCores = (batch b, T-half). Each core computes 512 query
rows with a 32-row halo; weights replicated.

Design: t-column-quarter pipeline so exp/shear/value/DMA-out overlap the
ACT tanh stream (ACT is the bottleneck engine: tanh at 1 col/cycle).
PE warm-up matmuls ramp the tensor engine to full clock before real work.
k is projected replicated-unshifted via a 4x-tiled Wx lhsT; the per-group
diagonal shift is applied by Pool group-sliced copies from a bf16 SBUF
image (GPSIMD cannot touch PSUM; PSUM egress is DVE/ACT-only). One-stage
radix-64 shear: 64 two-column PE shift-matmuls per quarter (matmul cost
scales with output columns only, so these are ~1ns each — no sigma
permutes needed, exp/Bsb stay in plain t-order). Softmax denominator via
ones-vector matmuls into a per-quarter PSUM column; recip + one
scale-copy per quarter on egress. PSUM deps are tile-coarse and tiles
are bank-granular (8 banks), so PSUM contents are grouped by readiness
epoch into bank-tiles with tag-rotation reuse. Input DMAs pipelined
smallest-first; per-DMA latency is ~2.2us (issue 565+625, DGE 650, sem
prop 900) so the first xt piece gates the stream start. tc.high_priority
pins the critical last-quarter chain against scheduler steals.
"""
import os
import sys

sys.path.insert(0, "/opt/trn_rl_repo")

import numpy as np
import ml_dtypes  # noqa: E402
import concourse.bass as bass  # noqa: E402
import concourse.mybir as mybir  # noqa: E402
from concourse import bacc, tile  # noqa: E402
from concourse.ap import AP  # noqa: E402
from concourse.bass_utils import run_bass_kernel_spmd  # noqa: E402

F32 = mybir.dt.float32
BF16 = mybir.dt.bfloat16
ActFn = mybir.ActivationFunctionType

B, T, D, U = 4, 1024, 512, 32
WIDTH = 64
T_LOC = 512
HALO = 576
NCORES = 8

_CDT = BF16 if os.environ.get("ATTN_CDT", "bf16") == "bf16" else F32
_VDT = BF16 if os.environ.get("ATTN_VOUT", "bf16") == "bf16" else F32

NWARM = int(os.environ.get("ATTN_NWARM", "5"))

# xt pieces: halo col windows (4-col overlaps so group-shifted reads stay
# within one piece) and q-projection t-windows fully inside each piece.
XPC = [(0, 196), (192, 324), (320, 452), (448, 576)]
QPC = [(0, 164), (164, 292), (292, 420), (420, 512)]

# ld column layout
W4X0 = 0                      # 4 chunks x 128 (Wx tiled 4x in M)
XTA = 512                     # xt piece A: 4 chunks x 196
W4T0 = XTA + 4 * 196          # Wt tiled: 512
XTB = W4T0 + 512              # 4 x 132
XTC = XTB + 4 * 132           # 4 x 132
XTD = XTC + 4 * 132           # 4 x 128
WA0 = XTD + 4 * 128           # wa_wide 124
SH0 = WA0 + 124               # sh 255
RLO = SH0 + 255               # rlo 32
RHI = RLO + 32                # rhi 32
ONE = RHI + 32                # ones column
LDC = ONE + 1

XTBASE = [XTA, XTB, XTC, XTD]


def _np_dt(cdt):
    return ml_dtypes.bfloat16 if cdt == BF16 else np.float32


def _emit(nc, tc, cdt, ld, xe, mbb, vout):
    from contextlib import ExitStack
    ctx = ExitStack()
    with ctx:
        cpool = ctx.enter_context(tc.tile_pool(name="consts", bufs=1))
        tpool = ctx.enter_context(tc.tile_pool(name="tanh", bufs=4))
        bpool = ctx.enter_context(tc.tile_pool(name="bsb", bufs=4))
        opool = ctx.enter_context(tc.tile_pool(name="outs", bufs=4))
        pp = ctx.enter_context(tc.tile_pool(name="pp", bufs=1, space="PSUM"))

        # ---------- t=0: ACT table preload + PE warmup ----------
        scratch = cpool.tile([128, 512], cdt, tag="scratch")
        nc.gpsimd.memset(scratch[:], 0.0)
        dummy = cpool.tile([1, 1], F32, tag="dummy")
        nc.vector.memset(dummy[:], 0.0)
        nc.scalar.activation(dummy[:], dummy[:], ActFn.Exp)

        B0 = cpool.tile([128, 512], cdt, tag="B0")
        nc.gpsimd.memset(B0[64:128, :], 0.0)

        warm_ps = pp.tile([128, 512], F32, tag="s1", name="warm_ps")
        for _ in range(NWARM):
            nc.tensor.matmul(warm_ps[:, 0:512], scratch[:, 0:128],
                             scratch[:], start=True, stop=True)

        # ---------- DMAs (SP queue, urgency order) ----------
        ld_sb = cpool.tile([128, LDC], cdt, tag="ld_sb")
        xe_sb = cpool.tile([128, 4096], cdt, tag="xe_sb")
        mbb_sb = cpool.tile([128, 2], F32, tag="mbb_sb")
        nc.gpsimd.dma_start(mbb_sb[:], mbb[:])
        for lo, hi in [(W4X0, XTA + 392), (XTA + 392, W4T0), (W4T0, XTB),
                       (XTB, XTC), (XTC, XTD), (XTD, WA0), (WA0, LDC)]:
            nc.sync.dma_start(ld_sb[:, lo:hi], ld[:, lo:hi])
        nc.sync.dma_start(xe_sb[:, 0:2048], xe[:, 0:2048])
        nc.sync.dma_start(xe_sb[:, 2048:4096], xe[:, 2048:4096])

        wa_sb = ld_sb[:, WA0:WA0 + 124]
        sh_sb = ld_sb[:, SH0:SH0 + 255]
        rlo_sb = ld_sb[0:32, RLO:RLO + 32]
        rhi_sb = ld_sb[0:32, RHI:RHI + 32]
        ones_sb = ld_sb[:, ONE:ONE + 1]
        bh4 = mbb_sb[:, 1:2]
        ba = mbb_sb[0:64, 0:1]

        def w4x(c):
            return ld_sb[:, W4X0 + 128 * c:W4X0 + 128 * c + 128]

        def w4t(c):
            return ld_sb[:, W4T0 + 128 * c:W4T0 + 128 * c + 128]

        def xp(p, c, a, b):
            # halo cols [a,b) of chunk c within xt piece p (local coords)
            base, w = XTBASE[p], XPC[p][1] - XPC[p][0]
            lo = XPC[p][0]
            return ld_sb[:, base + w * c + (a - lo):base + w * c + (b - lo)]

        # ---------- projections (PE) ----------
        # k4u_*[32g+u, j] = k[j, u] replicated (4x-tiled Wx lhsT);
        # q4*[32g+u, t] = q[t, u]
        k4uA = pp.tile([128, 196], F32, tag="s1", name="k4uA")
        k4uB = pp.tile([128, 132], F32, tag="s2", name="k4uB")
        k4uCD = pp.tile([128, 260], F32, tag="s3", name="k4uCD")
        q4A = pp.tile([128, 164], F32, tag="s4", name="q4A")
        q4B = pp.tile([128, 128], F32, tag="s5", name="q4B")
        q4CD = pp.tile([128, 220], F32, tag="s6", name="q4CD")

        for c in range(4):
            nc.tensor.matmul(k4uA[:], w4x(c), xp(0, c, 0, 196),
                             start=(c == 0), stop=(c == 3))
        for c in range(4):
            nc.tensor.matmul(q4A[:], w4t(c), xp(0, c, 32, 196),
                             start=(c == 0), stop=(c == 3))
        for c in range(4):
            nc.tensor.matmul(k4uB[:], w4x(c), xp(1, c, 192, 324),
                             start=(c == 0), stop=(c == 3))
        for c in range(4):
            nc.tensor.matmul(q4B[:], w4t(c), xp(1, c, 196, 324),
                             start=(c == 0), stop=(c == 3))
        for c in range(4):
            nc.tensor.matmul(k4uCD[:, 0:132], w4x(c), xp(2, c, 320, 452),
                             start=(c == 0), stop=(c == 3))
        for c in range(4):
            nc.tensor.matmul(q4CD[:, 0:128], w4t(c), xp(2, c, 324, 452),
                             start=(c == 0), stop=(c == 3))
        for c in range(4):
            nc.tensor.matmul(k4uCD[:, 132:260], w4x(c), xp(3, c, 448, 576),
                             start=(c == 0), stop=(c == 3))
        for c in range(4):
            nc.tensor.matmul(q4CD[:, 128:220], w4t(c), xp(3, c, 452, 544),
                             start=(c == 0), stop=(c == 3))

        # ---------- staging ----------
        k4 = cpool.tile([128, 576], cdt, tag="k4")
        k4u_sb = cpool.tile([128, 576], cdt, tag="k4u_sb")
        q4s = cpool.tile([128, 512], cdt, tag="q4s")

        # bf16 SBUF image of k4u (ACT/DVE egress); per-group shifted copies
        # k4u_sb -> k4 on Pool (SBUF->SBUF)
        with tc.high_priority():
            nc.scalar.copy(k4u_sb[:, 0:196], k4uA[:])

        def kgroup_pool(a, b):
            for g in range(4):
                nc.gpsimd.tensor_copy(k4[32 * g:32 * g + 32, a:b],
                                      k4u_sb[32 * g:32 * g + 32,
                                             a + g:b + g])

        # quarter 0 group copies: all on DVE (4x-mode SBUF copies, ~110ns
        # each — 3x faster than Pool), parallel with ACT's q4s staging
        with tc.high_priority():
            nc.vector.tensor_copy(k4[0:32, 0:192], k4u_sb[0:32, 0:192])
            nc.vector.tensor_copy(k4[32:64, 0:192], k4u_sb[32:64, 1:193])
            nc.vector.tensor_copy(k4[64:96, 0:192], k4u_sb[64:96, 2:194])
            nc.vector.tensor_copy(k4[96:128, 0:192], k4u_sb[96:128, 3:195])
        nc.scalar.copy(k4u_sb[:, 192:324], k4uB[:])
        nc.scalar.copy(q4s[:, 0:164], q4A[:])
        nc.scalar.copy(q4s[:, 164:292], q4B[:])
        nc.scalar.copy(q4s[:, 292:512], q4CD[:])
        kgroup_pool(192, 320)              # quarter 1

        # ---------- stream machinery ----------
        # tanh insts: q0/q1/q2 merged [128, 2048]; q3 as r-halves [128, 1024]
        touts = {}
        tins = {}

        def add_half(ti, qq, half, width):
            toff, r0 = 128 * qq, 8 * half
            if ti not in tins:
                tins[ti] = tpool.tile([128, width], cdt, tag="tin",
                                      name=f"tin{ti}")
            tin = tins[ti]
            coff = 1024 * half if width == 2048 else 0
            q_ap = AP(q4s[:].tensor, toff, [[512, 128], [0, 8], [1, 128]])
            k_ap = AP(k4[:].tensor, toff + 4 * r0,
                      [[576, 128], [4, 8], [1, 128]])
            o_ap = AP(tin[:].tensor, coff,
                      [[width, 128], [128, 8], [1, 128]])
            nc.vector.tensor_add(o_ap, q_ap, k_ap)

        def tanh_inst(ti, width):
            tout = tpool.tile([128, width], cdt, tag="tout",
                              name=f"tout{ti}")
            nc.scalar.activation(tout[:], tins[ti][:], ActFn.Tanh, bias=bh4)
            touts[ti] = tout

        # per-quarter PSUM bank: E [64, 0:128] | shear P [128, 128:256]
        # | denom col 256
        EP = {}

        def e_mms(ti, qq, r0, nr, coff=0):
            tout = touts[ti]
            for j in range(nr):
                r = r0 + j
                st = (r == 0)
                sp = (r == 15 and qq in (1, 2))
                nc.tensor.matmul(EP[qq][0:64, 0:128],
                                 wa_sb[:, 60 - 4 * r:124 - 4 * r],
                                 tout[:, coff + 128 * j:coff + 128 * j + 128],
                                 start=st, stop=sp)
            if r0 + nr == 16 and qq == 0:
                nc.tensor.matmul(EP[0][0:64, 0:32], sh_sb[0:32, 127:191],
                                 rlo_sb, start=False, stop=True)
            if r0 + nr == 16 and qq == 3:
                nc.tensor.matmul(EP[3][0:64, 96:128], sh_sb[0:32, 95:159],
                                 rhi_sb, start=False, stop=True)

        def exp_block(qq):
            e_in = AP(EP[qq][:].tensor, 0, [[512, 64], [1, 128]])
            b_out = AP(B0[:].tensor, 128 * qq, [[512, 64], [1, 128]])
            nc.scalar.activation(b_out, e_in, ActFn.Exp, bias=ba)

        def shear_q(qq):
            P_q = EP[qq]
            for tau in range(64):
                o_ap = AP(P_q[:].tensor, 128 + tau, [[512, 128], [64, 2]])
                r_ap = AP(B0[:].tensor, 128 * qq + tau,
                          [[512, 128], [64, 2]])
                nc.tensor.matmul(o_ap, sh_sb[:, 127 - tau:255 - tau], r_ap,
                                 start=True, stop=True)

        def value_q(qq, vp, bsb):
            for mp in range(2):
                lhsT = bsb[:, 64 * mp:64 * mp + 64]
                m = 2 * qq + mp
                nc.tensor.matmul(EP[qq][64 * mp:64 * mp + 64, 256:257],
                                 lhsT, ones_sb, start=True, stop=True)
                nc.tensor.matmul(vp[64 * mp:64 * mp + 64, 0:512], lhsT,
                                 xe_sb[:, 512 * m:512 * m + 512],
                                 start=True, stop=True)

        def finish_q(qq, vp, scale_eng, dma_eng=None, split=False):
            rc = opool.tile([128, 1], F32, tag="rc", name=f"rc{qq}")
            nc.vector.reciprocal(rc[:], EP[qq][:, 256:257])
            vs = opool.tile([128, 512], _VDT, tag="vs", name=f"vs{qq}")
            if split:
                # halve the scale latency: DVE and ACT each do 256 cols
                nc.vector.tensor_scalar_mul(vs[:, 0:256], vp[:, 0:256],
                                            rc[:])
                nc.scalar.activation(vs[:, 256:512], vp[:, 256:512],
                                     ActFn.Copy, scale=rc[:])
            elif scale_eng is nc.scalar:
                nc.scalar.activation(vs[:], vp[:], ActFn.Copy, scale=rc[:])
            else:
                scale_eng.tensor_scalar_mul(vs[:], vp[:], rc[:])
            (dma_eng or nc.sync).dma_start(
                vout[128 * qq:128 * qq + 128, :], vs[:])

        # ---- emission in global time order ----
        add_half(-1, 0, 0, 1024)           # DVE
        tanh_inst(-1, 1024)                # ACT: q0a
        add_half(0, 0, 1, 1024)
        tanh_inst(0, 1024)                 # q0b
        nc.vector.tensor_copy(k4u_sb[:, 324:452], k4uCD[:, 4:132])
        nc.vector.tensor_copy(k4u_sb[:, 452:575], k4uCD[:, 136:259])
        kgroup_pool(320, 448)              # quarter 2 (Pool)
        add_half(1, 1, 0, 2048)
        add_half(1, 1, 1, 2048)
        tanh_inst(1, 2048)                 # q1
        EP[0] = pp.tile([128, 512], F32, tag="s4", name="EP0")
        e_mms(-1, 0, 0, 8)                 # q0 r0-7
        e_mms(0, 0, 8, 8)                  # q0 r8-15 (incl rlo)
        kgroup_pool(448, 572)              # quarter 3 (Pool)
        exp_block(0)                       # ACT after tanh-q1
        add_half(2, 2, 0, 2048)
        add_half(2, 2, 1, 2048)
        tanh_inst(2, 2048)                 # q2
        EP[1] = pp.tile([128, 512], F32, tag="s2", name="EP1")
        e_mms(1, 1, 0, 16)
        shear_q(0)                         # PE after exp0
        bsb0 = bpool.tile([128, 128], cdt, tag="bsb", name="bsb0")
        nc.vector.tensor_copy(bsb0[:], EP[0][:, 128:256])
        vp0 = pp.tile([128, 512], F32, tag="s5", name="vp0")
        value_q(0, vp0, bsb0)
        finish_q(0, vp0, nc.vector)
        exp_block(1)                       # ACT after tanh-q2
        add_half(3, 3, 0, 1024)
        tanh_inst(3, 1024)                 # q3a
        EP[2] = pp.tile([128, 512], F32, tag="s6", name="EP2")
        e_mms(2, 2, 0, 16)
        shear_q(1)
        bsb1 = bpool.tile([128, 128], cdt, tag="bsb", name="bsb1")
        nc.vector.tensor_copy(bsb1[:], EP[1][:, 128:256])
        vp1 = pp.tile([128, 512], F32, tag="s7", name="vp1")
        value_q(1, vp1, bsb1)
        exp_block(2)                       # ACT after tanh-q3a
        add_half(4, 3, 1, 1024)
        with tc.high_priority():
            tout4 = tpool.tile([128, 512], cdt, tag="tout", name="tout4")
            nc.scalar.activation(tout4[:], tins[4][:, 0:512], ActFn.Tanh,
                                 bias=bh4)                 # q3b1 (r8-11)
            touts[4] = tout4
            EP[3] = pp.tile([128, 512], F32, tag="s3", name="EP3")
            e_mms(3, 3, 0, 8)              # q3 r0-7 (during tanh q3b)
        shear_q(2)
        bsb2 = bpool.tile([128, 128], cdt, tag="bsb", name="bsb2")
        nc.vector.tensor_copy(bsb2[:], EP[2][:, 128:256])
        finish_q(1, vp1, nc.vector)
        with tc.high_priority():
            tout5 = tpool.tile([128, 512], cdt, tag="tout", name="tout5")
            nc.scalar.activation(tout5[:], tins[4][:, 512:1024], ActFn.Tanh,
                                 bias=bh4)                 # q3b2 (r12-15)
            touts[5] = tout5
        vp2 = pp.tile([128, 512], F32, tag="s8", name="vp2")
        value_q(2, vp2, bsb2)
        with tc.high_priority():
            e_mms(4, 3, 8, 4)              # q3 r8-11 (during tanh q3b2)
        finish_q(2, vp2, nc.scalar)
        with tc.high_priority():
            e_mms(5, 3, 12, 4)             # q3 r12-15 + rhi
            exp_block(3)
            shear_q(3)
            bsb3 = bpool.tile([128, 128], cdt, tag="bsb", name="bsb3")
            nc.vector.tensor_copy(bsb3[:], EP[3][:, 128:256])
            vp3 = pp.tile([128, 512], F32, tag="s5", name="vp3")
            value_q(3, vp3, bsb3)
            finish_q(3, vp3, nc.vector)


def build_nc(cdt=_CDT):
    nc = bacc.Bacc("TRN2", target_bir_lowering=False)
    ld = nc.dram_tensor("ld", [128, LDC], cdt, kind="ExternalInput")
    xe = nc.dram_tensor("xe", [128, 4096], cdt, kind="ExternalInput")
    mbb = nc.dram_tensor("mbb", [128, 2], F32, kind="ExternalInput")
    vout = nc.dram_tensor("v", [T_LOC, D], _VDT, kind="ExternalOutput")
    with tile.TileContext(nc) as tc:
        _emit(nc, tc, cdt, ld, xe, mbb, vout)
    nc.compile()
    return nc


# ---------------- host-side prep ----------------

def prep_core_inputs(x, Wt, Wx, bh, Wa, ba, core, cdt=_CDT):
    ndt = _np_dt(cdt)
    b, half = core // 2, core % 2
    t0 = half * T_LOC
    lo, hi = t0 - 32, t0 + 544
    pad_lo, pad_hi = max(0, -lo), max(0, hi - T)
    xs = x[b, max(0, lo):min(T, hi), :]
    x_halo = np.pad(xs, ((pad_lo, pad_hi), (0, 0)))     # [576, 512]

    ld = np.zeros((128, LDC), np.float32)
    for c in range(4):
        xc = x_halo[:, 128 * c:128 * c + 128].T          # [128, 576]
        ld[:, W4X0 + 128 * c:W4X0 + 128 * c + 128] = \
            np.tile(Wx[128 * c:128 * c + 128, :], (1, 4))
        ld[:, W4T0 + 128 * c:W4T0 + 128 * c + 128] = \
            np.tile(Wt[128 * c:128 * c + 128, :], (1, 4))
        for p, (alo, ahi) in enumerate(XPC):
            w = ahi - alo
            ld[:, XTBASE[p] + w * c:XTBASE[p] + w * c + w] = xc[:, alo:ahi]
    for g in range(4):
        ld[32 * g:32 * g + 32, WA0 + 60 + g] = Wa[:, 0]
    kk = np.arange(128)
    ld[kk, SH0 + kk + 127] = 1.0
    ks = np.arange(32)[:, None]
    ts = np.arange(32)[None, :]
    if t0 == 0:
        ld[0:32, RLO:RLO + 32] = np.where(ts < 32 - ks, -30.0, 0.0)
    if t0 + T_LOC == T:
        ld[0:32, RHI:RHI + 32] = np.where((480 + ts) + (ks + 32) > 543,
                                          -30.0, 0.0)
    ld[:, ONE] = 1.0

    xe = np.empty((128, 4096), np.float32)
    for m in range(8):
        xe[:, 512 * m:512 * (m + 1)] = x_halo[64 * m:64 * m + 128, :]

    mbb = np.zeros((128, 2), np.float32)
    mbb[0:64, 0] = float(np.asarray(ba).reshape(-1)[0])
    mbb[:, 1] = np.tile(np.asarray(bh, np.float32), 4)

    return {"ld": ld.astype(ndt), "xe": xe.astype(ndt), "mbb": mbb}


_NC_CACHE = {}


def _get_nc(cdt=_CDT):
    key = str(cdt)
    if key not in _NC_CACHE:
        _NC_CACHE[key] = build_nc(cdt)
    return _NC_CACHE[key]


def kernel(x, Wt, Wx, bh, Wa, ba, _trace=False):
    x = np.asarray(x, np.float32)
    Wt = np.asarray(Wt, np.float32)
    Wx = np.asarray(Wx, np.float32)
    bh = np.asarray(bh, np.float32)
    Wa = np.asarray(Wa, np.float32)
    ba = np.asarray(ba, np.float32)
    nc = _get_nc()
    in_maps = [prep_core_inputs(x, Wt, Wx, bh, Wa, ba, c)
               for c in range(NCORES)]
    res = run_bass_kernel_spmd(nc, in_maps, core_ids=list(range(NCORES)),
                               trace=_trace)
    out = np.empty((B, T, D), np.float32)
    for c in range(NCORES):
        b, half = c // 2, c % 2
        out[b, half * T_LOC:(half + 1) * T_LOC, :] = np.asarray(
            res.results[c]["v"], np.float32)
    if _trace:
        return out, res
    return out
